# revision 4
# baseline (speedup 1.0000x reference)
"""Trainium2 Bass kernel for nn_NetGram (30-net grouped MLP + capsule routing).

Self-contained: hardcodes shapes from the problem spec. Shards batch B=8192
across 8 NeuronCores (1024 each); weights replicated.

Math (per sample b):
  h1 = relu(x @ W1[n] + b1[n])            n=30 nets, 784->20
  h2 = relu(h1 @ W2[n] + b2[n])           20->20
  u  = squash(h2)  == f[b,n] * h2  with f = sqrt(sq)/(1+sq), sq = sum_d h2^2
  priors[b,o,n,k] = u . R[o,n,:,k]  == f[b,n] * q[b,n,o,k],  q = h2r @ R
  3 routing iterations (softmax over o, squash over k).

The squash factor f commutes out of the d-contraction, so the tensor-engine
GEMMs run on raw relu(h2) and f is folded into the per-(b,n) routing weights.
"""
import sys
sys.path.insert(0, "/opt/trn_rl_repo")

import numpy as np
from contextlib import ExitStack

import concourse.bacc as bacc
import concourse.tile as tile
import concourse.mybir as mybir
from concourse.bass_utils import run_bass_kernel_spmd

F32 = mybir.dt.float32
F32R = mybir.dt.float32r
AF = mybir.ActivationFunctionType
ALU = mybir.AluOpType

B = 8192
NCORES = 8
BC = B // NCORES          # 1024 per core
IN = 784
INP = 896                 # padded to 7*128
NN, D, O, K = 30, 20, 10, 16
OK = O * K                # 160
ON = O * NN               # 300
NOK = NN * OK             # 4800
G6 = 5                    # h1 groups of 6 nets -> [120, 512] tiles
T3 = 10                   # h2 tiles of 3 nets  -> [96, 512] tiles

_cache = {}


def _prep_consts(W1, b1, W2, b2, R):
    """Host-side constant layout prep. R is route_weights [O, NN, D, K]."""
    # W1cat [896, 600]: col = g*120 + nl*20 + e  (net = 6g+nl)
    W1cat = np.zeros((INP, 600), np.float32)
    w = W1.transpose(1, 0, 2).reshape(IN, NN * D)  # [784, (n,e)]
    for g in range(G6):
        W1cat[:IN, g * 120:(g + 1) * 120] = w[:, g * 120:(g + 1) * 120]
    # b1t [120, 5]
    b1t = np.zeros((120, G6), np.float32)
    for g in range(G6):
        b1t[:, g] = b1[6 * g:6 * g + 6].reshape(120)
    # W2bd [120, 960]: out tile t (nets 3t..3t+2) from h1 group g=t//2
    W2bd = np.zeros((120, T3 * 96), np.float32)
    for t in range(T3):
        g = t // 2
        for ml in range(3):
            n = 3 * t + ml
            nl = n - 6 * g
            W2bd[nl * 20:nl * 20 + 20, t * 96 + ml * 32: t * 96 + ml * 32 + 20] = W2[n]
    # b2a [96, 10]
    b2a = np.zeros((96, T3), np.float32)
    for t in range(T3):
        for ml in range(3):
            b2a[ml * 32:ml * 32 + 20, t] = b2[3 * t + ml]
    # R3bd [96, 4800]: rhs for q-GEMM tile t; q layout (n, o, k) -> n*160+o*16+k
    R3bd = np.zeros((96, T3 * 480), np.float32)
    for t in range(T3):
        for ml in range(3):
            n = 3 * t + ml
            # [D, O*K]
            rn = R[:, n, :, :].transpose(1, 0, 2).reshape(D, OK)
            R3bd[ml * 32:ml * 32 + 20, t * 480 + ml * 160: t * 480 + (ml + 1) * 160] = rn
    # map96 [96, 300]: sq^T GEMM rhs; col space = per-tile 30 wide
    map96 = np.zeros((96, T3 * NN), np.float32)
    for t in range(T3):
        for ml in range(3):
            map96[ml * 32:ml * 32 + 20, t * NN + 3 * t + ml] = 1.0
    return W1cat, b1t, W2bd, b2a, R3bd, map96


def _build():
    nc = bacc.Bacc("TRN2", debug=False, num_devices=NCORES)
    xt_d = nc.dram_tensor("xt", [INP, BC], F32R, kind="ExternalInput").ap()
    w1_d = nc.dram_tensor("w1", [INP, 600], F32R, kind="ExternalInput").ap()
    b1_d = nc.dram_tensor("b1", [120, G6], F32, kind="ExternalInput").ap()
    w2_d = nc.dram_tensor("w2", [120, T3 * 96], F32R, kind="ExternalInput").ap()
    b2_d = nc.dram_tensor("b2", [96, T3], F32, kind="ExternalInput").ap()
    r3_d = nc.dram_tensor("r3", [96, T3 * 480], F32R, kind="ExternalInput").ap()
    mp_d = nc.dram_tensor("mp", [96, T3 * NN], F32R, kind="ExternalInput").ap()
    out_d = nc.dram_tensor("out", [BC, OK], F32, kind="ExternalOutput").ap()

    with tile.TileContext(nc) as tc, ExitStack() as ctx:
        cpool = ctx.enter_context(tc.tile_pool(name="consts", bufs=1))
        xpool = ctx.enter_context(tc.tile_pool(name="x", bufs=3))
        hpool = ctx.enter_context(tc.tile_pool(name="h", bufs=1))
        qpool = ctx.enter_context(tc.tile_pool(name="q", bufs=2))
        wpool = ctx.enter_context(tc.tile_pool(name="w", bufs=1))
        spool = ctx.enter_context(tc.tile_pool(name="smalls", bufs=2))
        ps_big = ctx.enter_context(tc.tile_pool(name="psb", bufs=3, space="PSUM"))
        ps_q = ctx.enter_context(tc.tile_pool(name="psq", bufs=2, space="PSUM"))
        ps_sq = ctx.enter_context(tc.tile_pool(name="pssq", bufs=2, space="PSUM"))

        # ---- load constants
        w1t = [cpool.tile([128, 600], F32R, tag=f"w1_{kc}", name=f"w1_{kc}") for kc in range(7)]
        for kc in range(7):
            nc.sync.dma_start(w1t[kc][:], w1_d[kc * 128:(kc + 1) * 128, :])
        b1t = cpool.tile([120, G6], F32, tag="b1", name="b1")
        nc.sync.dma_start(b1t[:], b1_d[:])
        w2t = cpool.tile([120, T3 * 96], F32R, tag="w2", name="w2")
        nc.sync.dma_start(w2t[:], w2_d[:])
        b2t = cpool.tile([96, T3], F32, tag="b2", name="b2")
        nc.sync.dma_start(b2t[:], b2_d[:])
        r3t = cpool.tile([96, T3 * 480], F32R, tag="r3", name="r3")
        nc.sync.dma_start(r3t[:], r3_d[:])
        mpt = cpool.tile([96, T3 * NN], F32R, tag="mp", name="mp")
        nc.sync.dma_start(mpt[:], mp_d[:])

        for bt in range(2):  # 512-wide batch tiles
            bo = bt * 512
            # ---- h1: 5 groups x [120, 512]
            h1r = []
            for g in range(G6):
                ps = ps_big.tile([128, 512], F32, tag="psbig", name="psbig")
                for kc in range(7):
                    xtile = xpool.tile([128, 512], F32R, tag="xt", name="xt")
                    nc.sync.dma_start(
                        xtile[:], xt_d[kc * 128:(kc + 1) * 128, bo:bo + 512])
                    nc.tensor.matmul(
                        ps[0:120, :], w1t[kc][:, g * 120:(g + 1) * 120],
                        xtile[:], start=(kc == 0), stop=(kc == 6))
                h = hpool.tile([120, 512], F32R, tag=f"h1r_{g}", name=f"h1r_{g}")
                nc.scalar.activation(h[:], ps[0:120, :], AF.Relu,
                                     bias=b1t[:, g:g + 1], scale=1.0)
                h1r.append(h)

            # ---- h2: 10 tiles x [96, 512] (3 nets x 32 rows)
            h2r, h2sq = [], []
            for t in range(T3):
                ps = ps_big.tile([128, 512], F32, tag="psbig", name="psbig")
                nc.tensor.matmul(ps[0:96, :], w2t[:, t * 96:(t + 1) * 96],
                                 h1r[t // 2][:], start=True, stop=True)
                hr = hpool.tile([96, 512], F32R, tag=f"h2r_{t}", name=f"h2r_{t}")
                nc.scalar.activation(hr[:], ps[0:96, :], AF.Relu,
                                     bias=b2t[:, t:t + 1], scale=1.0)
                h2r.append(hr)
                hs = hpool.tile([96, 512], F32R, tag=f"h2sq_{t}", name=f"h2sq_{t}")
                nc.scalar.activation(hs[:], hr[:], AF.Square)
                h2sq.append(hs)

            for sub in range(4):  # 128-wide routing tiles
                s0_ = sub * 128
                # ---- q [128, 4800] (layout n*160 + o*16 + k), q = h2r @ R
                qt = qpool.tile([128, NOK], F32, tag="q", name="q")
                for t in range(T3):
                    qps = ps_q.tile([128, 480], F32, tag="qps", name="qps")
                    nc.tensor.matmul(qps[:], h2r[t][:, s0_:s0_ + 128],
                                     r3t[:, t * 480:(t + 1) * 480],
                                     start=True, stop=True)
                    nc.scalar.copy(qt[:, t * 480:(t + 1) * 480], qps[:])
                # ---- sq[b, n] via map GEMM; f = sqrt(sq)/(1+sq)
                sqps = ps_sq.tile([128, NN], F32, tag="sqps", name="sqps")
                for t in range(T3):
                    nc.tensor.matmul(sqps[:], h2sq[t][:, s0_:s0_ + 128],
                                     mpt[:, t * NN:(t + 1) * NN],
                                     start=(t == 0), stop=(t == T3 - 1))
                sq = spool.tile([128, NN], F32, tag="sq", name="sq")
                nc.scalar.copy(sq[:], sqps[:])
                rt = spool.tile([128, NN], F32, tag="rt", name="rt")
                nc.scalar.activation(rt[:], sq[:], AF.Sqrt)
                dd = spool.tile([128, NN], F32, tag="dd", name="dd")
                nc.vector.tensor_scalar_add(dd[:], sq[:], 1.0)
                di = spool.tile([128, NN], F32, tag="di", name="di")
                nc.vector.reciprocal(di[:], dd[:])
                ff = spool.tile([128, NN], F32, tag="ff", name="ff")
                nc.vector.tensor_tensor(ff[:], rt[:], di[:], op=ALU.mult)

                # views of q
                q_nok = qt[:].rearrange("p (n o k) -> p n o k", n=NN, o=O, k=K)

                wt = wpool.tile([128, NOK], F32, tag="wt", name="wt")
                w_nok = wt[:].rearrange("p (n o k) -> p n o k", n=NN, o=O, k=K)
                w_okn = wt[:].rearrange("p (n o k) -> p o k n", n=NN, o=O, k=K)
                w_nok2 = wt[:].rearrange("p (n o k) -> p (n o) k", n=NN, o=O, k=K)

                def bcast_no(t128):  # [128, NN] -> (n, o, k) bcast over o,k
                    return t128[:, :, None, None].broadcast_to([128, NN, O, K])

                def bcast_ok(t160):  # [128, OK] -> (n, o, k) bcast over n
                    return t160.rearrange("p (o k) -> p o k", o=O, k=K)[
                        :, None, :, :].broadcast_to([128, NN, O, K])

                def bcast_no2(t300):  # [128, ON] (n,o) -> bcast over k
                    return t300.rearrange("p (n o) -> p n o", n=NN, o=O)[
                        :, :, :, None].broadcast_to([128, NN, O, K])

                def s_pass(weights_no, tag):
                    """s[b,(o,k)] = sum_n weights[b,n,o] * q[b,n,o,k]"""
                    nc.vector.tensor_tensor(w_nok, q_nok, weights_no, op=ALU.mult)
                    s = spool.tile([128, OK], F32, tag=tag, name=tag)
                    nc.vector.tensor_reduce(
                        s[:].rearrange("p (o k) -> p o k", o=O, k=K), w_okn,
                        axis=mybir.AxisListType.X, op=ALU.add)
                    return s

                def a_pass(v160, tag):
                    """A[b,(n,o)] = sum_k q[b,n,o,k] * v[b,o,k]"""
                    nc.vector.tensor_tensor(w_nok, q_nok, bcast_ok(v160[:]), op=ALU.mult)
                    a = spool.tile([128, ON], F32, tag=tag, name=tag)
                    nc.vector.tensor_reduce(
                        a[:], w_nok2, axis=mybir.AxisListType.X, op=ALU.add)
                    return a

                def squash_gamma(s, sq_scale, tag):
                    """gamma' = sqrt(sq)/(1+sq_scale*sq); sq = sum_k s^2 (raw)."""
                    ss = spool.tile([128, OK], F32, tag=tag + "_ss", name=tag + "_ss")
                    nc.vector.tensor_tensor(ss[:], s[:], s[:], op=ALU.mult)
                    sqv = spool.tile([128, O], F32, tag=tag + "_sq", name=tag + "_sq")
                    nc.vector.tensor_reduce(
                        sqv[:], ss[:].rearrange("p (o k) -> p o k", o=O, k=K),
                        axis=mybir.AxisListType.X, op=ALU.add)
                    r_ = spool.tile([128, O], F32, tag=tag + "_r", name=tag + "_r")
                    nc.scalar.activation(r_[:], sqv[:], AF.Sqrt)
                    d_ = spool.tile([128, O], F32, tag=tag + "_d", name=tag + "_d")
                    nc.vector.tensor_scalar(
                        out=d_[:], in0=sqv[:], scalar1=sq_scale, scalar2=1.0,
                        op0=ALU.mult, op1=ALU.add)
                    di_ = spool.tile([128, O], F32, tag=tag + "_di", name=tag + "_di")
                    nc.vector.reciprocal(di_[:], d_[:])
                    g_ = spool.tile([128, O], F32, tag=tag + "_g", name=tag + "_g")
                    nc.vector.tensor_tensor(g_[:], r_[:], di_[:], op=ALU.mult)
                    return g_

                def bcast_g(g10):  # [128, O] -> (o, k) bcast over k
                    return g10[:, :, None].broadcast_to([128, O, K])

                def softmax_probs(logits, tag):
                    """probs' = exp(logits)/Z * f  per (b, n); logits [128,(n,o)]"""
                    e = spool.tile([128, ON], F32, tag=tag + "_e", name=tag + "_e")
                    nc.scalar.activation(e[:], logits[:], AF.Exp)
                    z = spool.tile([128, NN], F32, tag=tag + "_z", name=tag + "_z")
                    nc.vector.tensor_reduce(
                        z[:], e[:].rearrange("p (n o) -> p n o", n=NN, o=O),
                        axis=mybir.AxisListType.X, op=ALU.add)
                    iz = spool.tile([128, NN], F32, tag=tag + "_iz", name=tag + "_iz")
                    nc.vector.reciprocal(iz[:], z[:])
                    izf = spool.tile([128, NN], F32, tag=tag + "_izf", name=tag + "_izf")
                    nc.vector.tensor_tensor(izf[:], iz[:], ff[:], op=ALU.mult)
                    p = spool.tile([128, ON], F32, tag=tag + "_p", name=tag + "_p")
                    e_v = e[:].rearrange("p (n o) -> p n o", n=NN, o=O)
                    p_v = p[:].rearrange("p (n o) -> p n o", n=NN, o=O)
                    izf_b = izf[:, :, None].broadcast_to([128, NN, O])
                    nc.vector.tensor_tensor(p_v, e_v, izf_b, op=ALU.mult)
                    return p

                # ---- iteration 0: probs = 1/10 -> s0 = 0.1 * sum_n f*q
                s0t = s_pass(bcast_no(ff[:]), "s0")
                g0 = squash_gamma(s0t, 0.01, "g0")
                v0 = spool.tile([128, OK], F32, tag="v0", name="v0")
                nc.vector.scalar_tensor_tensor(
                    out=v0[:].rearrange("p (o k) -> p o k", o=O, k=K),
                    in0=s0t[:].rearrange("p (o k) -> p o k", o=O, k=K),
                    scalar=0.01, in1=bcast_g(g0[:]), op0=ALU.mult, op1=ALU.mult)

                # ---- A0 = (q . v0) * f ; logits1 = A0
                a0q = a_pass(v0, "a0q")
                a0 = spool.tile([128, ON], F32, tag="a0", name="a0")
                nc.vector.tensor_tensor(
                    a0[:].rearrange("p (n o) -> p n o", n=NN, o=O),
                    a0q[:].rearrange("p (n o) -> p n o", n=NN, o=O),
                    ff[:, :, None].broadcast_to([128, NN, O]), op=ALU.mult)

                # ---- iteration 1
                p1 = softmax_probs(a0, "p1")
                s1t = s_pass(bcast_no2(p1[:]), "s1")
                g1 = squash_gamma(s1t, 1.0, "g1")
                v1 = spool.tile([128, OK], F32, tag="v1", name="v1")
                nc.vector.tensor_tensor(
                    v1[:].rearrange("p (o k) -> p o k", o=O, k=K),
                    s1t[:].rearrange("p (o k) -> p o k", o=O, k=K),
                    bcast_g(g1[:]), op=ALU.mult)

                # logits2 = A0 + (q . v1) * f
                a1q = a_pass(v1, "a1q")
                l2 = spool.tile([128, ON], F32, tag="l2", name="l2")
                nc.vector.tensor_tensor(
                    l2[:].rearrange("p (n o) -> p n o", n=NN, o=O),
                    a1q[:].rearrange("p (n o) -> p n o", n=NN, o=O),
                    ff[:, :, None].broadcast_to([128, NN, O]), op=ALU.mult)
                nc.vector.tensor_tensor(l2[:], l2[:], a0[:], op=ALU.add)

                # ---- iteration 2
                p2 = softmax_probs(l2, "p2")
                s2t = s_pass(bcast_no2(p2[:]), "s2")
                g2 = squash_gamma(s2t, 1.0, "g2")
                v2 = spool.tile([128, OK], F32, tag="v2", name="v2")
                nc.vector.tensor_tensor(
                    v2[:].rearrange("p (o k) -> p o k", o=O, k=K),
                    s2t[:].rearrange("p (o k) -> p o k", o=O, k=K),
                    bcast_g(g2[:]), op=ALU.mult)

                nc.sync.dma_start(out_d[bo + s0_:bo + s0_ + 128, :], v2[:])

    nc.compile()
    return nc


def kernel(x, W1, b1, W2, b2, route_weights):
    x = np.asarray(x, np.float32)
    W1 = np.asarray(W1, np.float32)
    b1 = np.asarray(b1, np.float32)
    W2 = np.asarray(W2, np.float32)
    b2 = np.asarray(b2, np.float32)
    R = np.asarray(route_weights, np.float32)

    if "nc" not in _cache:
        _cache["nc"] = _build()
    nc = _cache["nc"]

    W1cat, b1t, W2bd, b2a, R3bd, map96 = _prep_consts(W1, b1, W2, b2, R)
    xtp = np.zeros((INP, B), np.float32)
    xtp[:IN, :] = np.ascontiguousarray(x.T)

    in_maps = []
    for c in range(NCORES):
        in_maps.append({
            "xt": np.ascontiguousarray(xtp[:, c * BC:(c + 1) * BC]),
            "w1": W1cat, "b1": b1t, "w2": W2bd, "b2": b2a,
            "r3": R3bd, "mp": map96,
        })
    res = run_bass_kernel_spmd(nc, in_maps, list(range(NCORES)))
    _cache["last_res"] = res
    out = np.concatenate([res.results[c]["out"] for c in range(NCORES)], axis=0)
    return out.reshape(B, O, K)
